# revision 8
# baseline (speedup 1.0000x reference)
"""BCQLinear (3-bit binary-coded quantized linear) Trainium2 kernel.

Full-input contract: kernel(**inputs) takes the unsharded inputs of
nn_BCQLinear_88510686036218 and returns the full [1, 128, 4096] output.

Math: w = alpha*(2*S-7) + beta with S in [0,8) the 3-bit code, then
y = (x[:, in_reorder] @ w)[:, out_reorder].
Rewritten: w = A2*V + B with V = S-4 in [-4,4), A2 = 2*alpha, B = alpha+beta.

Sharding: out-features split 8 ways (512 cols/core), x replicated.

Per-core device program (SPMD, one Bass program):
  - Contraction rows are band-packed: K-tile kt in [0,32), partition p:
    dequant row i(kt,p) = 128*(p//4) + 4*kt + (p%4), so a single [128,512]
    alpha tile (A2_rep[p,:] = A2[p//4,:]) serves every K-tile.
  - Codes arrive as packed int32 words: 8 fields of 3 bits at bits
    [29-3r, 32-3r), biased so field value F = (S+4)&7 decodes to V = S-4
    via one tensor_scalar op: V = (W << 3r) >>(arith) 29.
  - 8 tensor_scalar unpacks -> V [128, 16384] fp32, then per K-tile a
    tensor_tensor multiply by A2_rep -> Wm, matmul-accumulated into PSUM
    [t=128, o'=512].  Per-group x sums (for the +B part) come from a
    G_sel indicator matmul; one final K=32 matmul adds xsumT.T @ B_mat.
"""
import numpy as np
from contextlib import ExitStack

import concourse.bass as bass
import concourse.mybir as mybir
import concourse.tile as tile
from concourse import bacc

IN_F, OUT_F, WBITS, GS, OFI = 4096, 4096, 3, 128, 128
NG, NB = 32, 32
NCORES = 8
OPC = OUT_F // NCORES        # 512 out-cols per core
NKT = 32                     # K-tiles of 128 rows
NF = 8                       # 3-bit fields packed per int32 word
NWC = OPC // NF              # 64 words per (partition, K-tile)
T = 128                      # tokens

F32 = mybir.dt.float32
I32 = mybir.dt.int32
ALU = mybir.AluOpType

_PROGRAM_CACHE = {}


# ---------------------------------------------------------------- host prep
def _dequant_codes(qweight):
    """S[i, o] in [0,8): w = alpha*(2S-7)+beta."""
    qw = np.asarray(qweight, dtype=np.uint32).reshape(NG, NB, WBITS, GS * OFI // 32)
    bits = (qw[..., None] >> np.arange(32, dtype=np.uint32)) & 1
    bits = bits.reshape(NG, NB, WBITS, GS, OFI)
    S = (bits * (2 ** np.arange(WBITS, dtype=np.uint32))[:, None, None]).sum(axis=2)
    return S.transpose(0, 2, 1, 3).reshape(IN_F, OUT_F).astype(np.int32)


def _band_rows():
    kt, p = np.meshgrid(np.arange(NKT), np.arange(128), indexing="ij")
    return 128 * (p // 4) + 4 * kt + (p % 4)      # [NKT, 128]


def _prepare(inputs):
    x = np.asarray(inputs["x"], np.float32).reshape(-1, IN_F)
    alpha = np.asarray(inputs["alpha"], np.float32)
    beta = np.asarray(inputs["beta"], np.float32)
    in_reorder = np.asarray(inputs["in_reorder"], np.int64)
    xf = x[:, in_reorder]

    S = _dequant_codes(inputs["qweight"])
    A2full = (2.0 * alpha).astype(np.float32)
    Bfull = (alpha + beta).astype(np.float32)

    rows = _band_rows()                            # [NKT, 128]
    XT = np.ascontiguousarray(
        xf[:, rows.reshape(-1)].T.reshape(NKT, 128, T).transpose(1, 0, 2)
    ).reshape(128, NKT * T)                        # [p, kt*T]

    shifts = (29 - 3 * np.arange(NF, dtype=np.uint32))[None, None, :, None]
    in_maps = []
    for c in range(NCORES):
        cols = slice(OPC * c, OPC * (c + 1))
        Vc = S[rows.reshape(-1), cols].reshape(NKT, 128, OPC).transpose(1, 0, 2)
        Fb = (Vc ^ 4).astype(np.uint32).reshape(128, NKT, NF, NWC)
        W = (Fb << shifts).sum(axis=2, dtype=np.uint64).astype(np.uint32)
        in_maps.append(dict(
            xt=XT,
            w=np.ascontiguousarray(W.reshape(128, NKT * NWC)).view(np.int32),
            a2=np.ascontiguousarray(A2full[np.arange(128) // 4][:, cols]),
            bmat=np.ascontiguousarray(Bfull[:, cols]),
            gsel=(np.arange(128)[:, None] // 4 == np.arange(32)[None, :]
                  ).astype(np.float32),
        ))
    return in_maps


# ---------------------------------------------------------------- program
def build_program():
    nc = bacc.Bacc("TRN2")
    xt = nc.declare_dram_parameter("xt", [128, NKT * T], F32, isOutput=False)
    w = nc.declare_dram_parameter("w", [128, NKT * NWC], I32, isOutput=False)
    a2 = nc.declare_dram_parameter("a2", [128, OPC], F32, isOutput=False)
    bmat = nc.declare_dram_parameter("bmat", [NG, OPC], F32, isOutput=False)
    gsel = nc.declare_dram_parameter("gsel", [128, NG], F32, isOutput=False)
    z = nc.declare_dram_parameter("z", [T, OPC], F32, isOutput=True)

    with tile.TileContext(nc) as tc, ExitStack() as ctx:
        cpool = ctx.enter_context(tc.tile_pool(name="const", bufs=1))
        vpool = ctx.enter_context(tc.tile_pool(name="vbuf", bufs=1))
        wmpool = ctx.enter_context(tc.tile_pool(name="wm", bufs=4))
        opool = ctx.enter_context(tc.tile_pool(name="out", bufs=1))
        ppool = ctx.enter_context(tc.tile_pool(name="psum", bufs=1, space="PSUM"))

        xt_sb = cpool.tile([128, NKT * T], F32, tag="xt")
        w_sb = cpool.tile([128, NKT * NWC], I32, tag="w")
        a2_sb = cpool.tile([128, OPC], F32, tag="a2")
        bm_sb = cpool.tile([NG, OPC], F32, tag="bm")
        gs_sb = cpool.tile([128, NG], F32, tag="gs")
        nc.sync.dma_start(out=w_sb[:], in_=w[:])
        nc.sync.dma_start(out=xt_sb[:], in_=xt[:])
        nc.sync.dma_start(out=a2_sb[:], in_=a2[:])
        nc.sync.dma_start(out=bm_sb[:], in_=bmat[:])
        nc.sync.dma_start(out=gs_sb[:], in_=gsel[:])

        # V[p, r*2048 + kt*64 + c] = (W[p, kt*64+c] << 3r) >>a 29  (int32;
        # bitvec ops cannot cast on write — the cast rides the mult below)
        v_sb = vpool.tile([128, NF * NKT * NWC], I32, tag="v")
        for r in range(NF):
            nc.vector.tensor_scalar(
                v_sb[:, r * NKT * NWC:(r + 1) * NKT * NWC],
                w_sb[:],
                3 * r,
                29,
                ALU.logical_shift_left,
                ALU.arith_shift_right,
            )
        # view of V as [p, kt, r, c] for per-K-tile reads
        v4 = v_sb[:].rearrange("p (r kt c) -> p kt r c", r=NF, kt=NKT, c=NWC)

        psum_main = ppool.tile([T, OPC], F32, tag="main")
        psum_xs = ppool.tile([NG, T], F32, tag="xs")
        for kt in range(NKT):
            wm = wmpool.tile([128, OPC], F32, tag="wm")
            nc.vector.tensor_tensor(
                wm[:].rearrange("p (r c) -> p r c", r=NF),
                v4[:, kt],
                a2_sb[:].rearrange("p (r c) -> p r c", r=NF),
                ALU.mult,
            )
            nc.tensor.matmul(
                psum_main[:],
                xt_sb[:, kt * T:(kt + 1) * T],
                wm[:],
                start=(kt == 0),
                stop=False,
            )
            nc.tensor.matmul(
                psum_xs[:],
                gs_sb[:],
                xt_sb[:, kt * T:(kt + 1) * T],
                start=(kt == 0),
                stop=(kt == NKT - 1),
            )
        xs_sb = opool.tile([NG, T], F32, tag="xs_sb")
        nc.vector.tensor_copy(xs_sb[:], psum_xs[:])
        nc.tensor.matmul(psum_main[:], xs_sb[:], bm_sb[:], start=False, stop=True)

        out_sb = opool.tile([T, OPC], F32, tag="out_sb")
        nc.vector.tensor_copy(out_sb[:], psum_main[:])
        nc.sync.dma_start(out=z[:], in_=out_sb[:])
    nc.finalize()
    return nc


def _get_program():
    if "nc" not in _PROGRAM_CACHE:
        _PROGRAM_CACHE["nc"] = build_program()
    return _PROGRAM_CACHE["nc"]


# ---------------------------------------------------------------- entry
def kernel(**inputs):
    from concourse.bass_utils import run_bass_kernel_spmd

    in_maps = _prepare(inputs)
    nc = _get_program()
    res = run_bass_kernel_spmd(nc, in_maps, list(range(NCORES)))
    z = np.concatenate([res.results[c]["z"] for c in range(NCORES)], axis=1)
    out_reorder = np.asarray(inputs["out_reorder"], np.int64)
    y = z[:, out_reorder].reshape(1, T, OUT_F).astype(np.float32)
    return y


# revision 13
# speedup vs baseline: 1.3427x; 1.3427x over previous
"""BCQLinear (3-bit binary-coded quantized linear) Trainium2 kernel.

Full-input contract: kernel(**inputs) takes the unsharded inputs of
nn_BCQLinear_88510686036218 and returns the full [1, 128, 4096] output.

Math: w = alpha*(2*S-7) + beta with S in [0,8) the 3-bit code, then
y = (x[:, in_reorder] @ w)[:, out_reorder].
Rewritten: w = A2*V + B with V = S-4 in [-4,4), A2 = 2*alpha, B = alpha+beta.

Sharding: out-features split 8 ways (512 cols/core), x replicated.

Per-core device program (SPMD, one Bass program):
  - Contraction rows are band-packed: K-tile kt in [0,32), partition p:
    dequant row i(kt,p) = 128*(p//4) + 4*kt + (p%4), so a single [128,512]
    alpha tile (A2_rep[p,:] = A2[p//4,:]) serves every K-tile.
  - Codes arrive as packed int32 words: 8 fields of 3 bits at bits
    [29-3r, 32-3r), biased so field value F = (S+4)&7 decodes to V = S-4
    via one two-op tensor_scalar: V = (W << 3r) >>(arith) 29.
  - Work is split into NCHUNK K-tile chunks so DMA / unpack (DVE) /
    dequant-mult (DVE) / matmul (PE) pipeline across chunks.
  - The beta part is y += xsum @ B with xsum[g,t] the per-group sums of
    permuted x (host-computed; 0.01% of the FLOPs) via one K=32 matmul.
"""
import numpy as np
from contextlib import ExitStack

import concourse.bass as bass
import concourse.mybir as mybir
import concourse.tile as tile
from concourse import bacc

IN_F, OUT_F, WBITS, GS, OFI = 4096, 4096, 3, 128, 128
NG, NB = 32, 32
NCORES = 8
OPC = OUT_F // NCORES        # 512 out-cols per core
NKT = 32                     # K-tiles of 128 rows
NF = 8                       # 3-bit fields packed per int32 word
NWC = OPC // NF              # 64 words per (partition, K-tile)
T = 128                      # tokens
NCHUNK = 4                   # pipeline chunks
KTC = NKT // NCHUNK          # K-tiles per chunk

F32 = mybir.dt.float32
I32 = mybir.dt.int32
ALU = mybir.AluOpType

_PROGRAM_CACHE = {}


# ---------------------------------------------------------------- host prep
def _dequant_codes(qweight):
    """S[i, o] in [0,8): w = alpha*(2S-7)+beta."""
    qw = np.asarray(qweight, dtype=np.uint32).reshape(NG, NB, WBITS, GS * OFI // 32)
    bits = (qw[..., None] >> np.arange(32, dtype=np.uint32)) & 1
    bits = bits.reshape(NG, NB, WBITS, GS, OFI)
    S = (bits * (2 ** np.arange(WBITS, dtype=np.uint32))[:, None, None]).sum(axis=2)
    return S.transpose(0, 2, 1, 3).reshape(IN_F, OUT_F).astype(np.int32)


def _band_rows():
    kt, p = np.meshgrid(np.arange(NKT), np.arange(128), indexing="ij")
    return 128 * (p // 4) + 4 * kt + (p % 4)      # [NKT, 128]


def _prepare(inputs):
    x = np.asarray(inputs["x"], np.float32).reshape(-1, IN_F)
    alpha = np.asarray(inputs["alpha"], np.float32)
    beta = np.asarray(inputs["beta"], np.float32)
    in_reorder = np.asarray(inputs["in_reorder"], np.int64)
    xf = x[:, in_reorder]

    S = _dequant_codes(inputs["qweight"])
    A2full = (2.0 * alpha).astype(np.float32)
    Bfull = (alpha + beta).astype(np.float32)

    rows = _band_rows()                            # [NKT, 128]
    XT = np.ascontiguousarray(
        xf[:, rows.reshape(-1)].T.reshape(NKT, 128, T).transpose(1, 0, 2)
    ).reshape(128, NKT * T)                        # [p, kt*T]
    # per-group token sums (beta part): xsumT[g, t]
    xsumT = np.ascontiguousarray(
        xf.reshape(T, NG, GS).sum(axis=2, dtype=np.float64).T.astype(np.float32)
    )

    shifts = (29 - 3 * np.arange(NF, dtype=np.uint32))[None, None, :, None]
    in_maps = []
    for c in range(NCORES):
        cols = slice(OPC * c, OPC * (c + 1))
        Vc = S[rows.reshape(-1), cols].reshape(NKT, 128, OPC).transpose(1, 0, 2)
        Fb = (Vc ^ 4).astype(np.uint32).reshape(128, NKT, NF, NWC)
        W = (Fb << shifts).sum(axis=2, dtype=np.uint64).astype(np.uint32)
        in_maps.append(dict(
            xt=XT,
            w=np.ascontiguousarray(W.reshape(128, NKT * NWC)).view(np.int32),
            a2=np.ascontiguousarray(A2full[np.arange(128) // 4][:, cols]),
            bmat=np.ascontiguousarray(Bfull[:, cols]),
            xsumt=xsumT,
        ))
    return in_maps


# ---------------------------------------------------------------- program
def build_program():
    nc = bacc.Bacc("TRN2")
    xt = nc.declare_dram_parameter("xt", [128, NKT * T], F32, isOutput=False)
    w = nc.declare_dram_parameter("w", [128, NKT * NWC], I32, isOutput=False)
    a2 = nc.declare_dram_parameter("a2", [128, OPC], F32, isOutput=False)
    bmat = nc.declare_dram_parameter("bmat", [NG, OPC], F32, isOutput=False)
    xsumt = nc.declare_dram_parameter("xsumt", [NG, T], F32, isOutput=False)
    z = nc.declare_dram_parameter("z", [T, OPC], F32, isOutput=True)

    CW = KTC * NWC            # words per chunk per partition (512)
    CX = KTC * T              # x cols per chunk per partition (1024)

    with tile.TileContext(nc) as tc, ExitStack() as ctx:
        cpool = ctx.enter_context(tc.tile_pool(name="const", bufs=1))
        wmpool = ctx.enter_context(tc.tile_pool(name="wm", bufs=4))
        opool = ctx.enter_context(tc.tile_pool(name="out", bufs=1))
        ppool = ctx.enter_context(tc.tile_pool(name="psum", bufs=1, space="PSUM"))

        a2_sb = cpool.tile([128, OPC], F32, tag="a2")
        bm_sb = cpool.tile([NG, OPC], F32, tag="bm")
        xs_sb = cpool.tile([NG, T], F32, tag="xs")
        nc.sync.dma_start(out=a2_sb[:], in_=a2[:])
        nc.sync.dma_start(out=bm_sb[:], in_=bmat[:])
        nc.sync.dma_start(out=xs_sb[:], in_=xsumt[:])

        w_sb, xt_sb, v_sb = [], [], []
        for ch in range(NCHUNK):
            wt = cpool.tile([128, CW], I32, tag=f"w{ch}", name=f"w{ch}")
            nc.sync.dma_start(out=wt[:], in_=w[:, ch * CW:(ch + 1) * CW])
            w_sb.append(wt)
            xtt = cpool.tile([128, CX], F32, tag=f"xt{ch}", name=f"xt{ch}")
            nc.sync.dma_start(out=xtt[:], in_=xt[:, ch * CX:(ch + 1) * CX])
            xt_sb.append(xtt)
            v_sb.append(cpool.tile([128, NF * CW], I32, tag=f"v{ch}", name=f"v{ch}"))

        psum_main = ppool.tile([T, OPC], F32, tag="main")
        for ch in range(NCHUNK):
            # unpack: V[p, r*CW + kt*64 + c] = (W[p, r-block] << 3r) >>a 29
            for r in range(NF):
                nc.vector.tensor_scalar(
                    v_sb[ch][:, r * CW:(r + 1) * CW],
                    w_sb[ch][:],
                    3 * r,
                    29,
                    ALU.logical_shift_left,
                    ALU.arith_shift_right,
                )
            v4 = v_sb[ch][:].rearrange("p (r kt c) -> p kt r c", r=NF, kt=KTC, c=NWC)
            for k in range(KTC):
                kt = ch * KTC + k
                wm = wmpool.tile([128, OPC], F32, tag="wm")
                nc.vector.tensor_tensor(
                    wm[:].rearrange("p (r c) -> p r c", r=NF),
                    v4[:, k],
                    a2_sb[:].rearrange("p (r c) -> p r c", r=NF),
                    ALU.mult,
                )
                nc.tensor.matmul(
                    psum_main[:],
                    xt_sb[ch][:, k * T:(k + 1) * T],
                    wm[:],
                    start=(kt == 0),
                    stop=False,
                )
        nc.tensor.matmul(psum_main[:], xs_sb[:], bm_sb[:], start=False, stop=True)

        out_sb = opool.tile([T, OPC], F32, tag="out_sb")
        nc.vector.tensor_copy(out_sb[:], psum_main[:])
        nc.sync.dma_start(out=z[:], in_=out_sb[:])
    nc.finalize()
    return nc


def _get_program():
    if "nc" not in _PROGRAM_CACHE:
        _PROGRAM_CACHE["nc"] = build_program()
    return _PROGRAM_CACHE["nc"]


# ---------------------------------------------------------------- entry
def kernel(**inputs):
    from concourse.bass_utils import run_bass_kernel_spmd

    in_maps = _prepare(inputs)
    nc = _get_program()
    res = run_bass_kernel_spmd(nc, in_maps, list(range(NCORES)))
    z = np.concatenate([res.results[c]["z"] for c in range(NCORES)], axis=1)
    out_reorder = np.asarray(inputs["out_reorder"], np.int64)
    y = z[:, out_reorder].reshape(1, T, OUT_F).astype(np.float32)
    return y
